# revision 17
# baseline (speedup 1.0000x reference)
"""CrossViewContrastiveLoss Trainium2 kernel.

loss = f(v1^T @ v2) where v1, v2 are [131072, 256] fp32 and f is a cheap
normalize/log epilogue on the [256, 256] joint matrix.

Strategy (data-parallel over N across 8 cores):
  - core c computes partial_c = v1[rows]^T @ v2[rows] for its 16384-row
    shard as a PE GEMM streaming 32 MiB of HBM (memory-bound).
  - the shard is viewed as [4096, 1024] so every DMA descriptor line is
    4 KiB contiguous (4x fewer descriptors than the [16384, 256] view);
    each 128-partition "flat tile" then holds 4 sub-rows per partition
    and contributes 4 rank-128 matmul groups.
  - chunk schedule: tiny first chunk (descriptor gen off the critical
    path at stream start), big middle chunks, descending tail so the
    last-arriving bytes have minimal downstream compute.
  - host sums the eight 256x256 partials in float64 and runs the epilogue
    (65536 elements -- negligible next to 256 MiB of streaming).
"""

import os

import numpy as np

import concourse.bacc as bacc
import concourse.bass as bass
import concourse.mybir as mybir
import concourse.tile as tile
from concourse import bass_utils

N_FULL = 131072
K = 256
NCORES = 8
N_LOC = N_FULL // NCORES  # 16384 rows per core
P = 128
RPP = int(os.environ.get("CVCL_RPP", "4"))  # rows packed per partition line
M = RPP * K  # 1024 elements per flat row
NF = N_LOC // RPP  # 4096 flat rows per core
NT = NF // P  # 32 flat tiles of 128 flat rows
CHUNK = int(os.environ.get("CVCL_CHUNK", "2"))  # flat tiles per DMA (max)
ALPHA = 9.0
EPS = 2.220446049250313e-16

_BUILD_CACHE = {}
LAST_RESULT = None  # BassKernelResults of the most recent run (for test.py)


def _install_axon_hooks_shim():
    """bass_utils' trace path imports antenv.axon_hooks, which this image
    lacks. Provide it, wiring the ctypes NTFF hook from trn_boot when the
    axon .so supports it. Harmless no-op when tracing is off."""
    import sys
    import types

    try:
        from antenv import axon_hooks  # noqa: F401

        return
    except ImportError:
        pass
    try:
        import antenv
    except ImportError:
        return
    mod = types.ModuleType("antenv.axon_hooks")
    mod._hook = None
    mod._resolved = False

    def set_axon_ntff_profile_hook(h):
        mod._hook = h
        mod._resolved = True

    def get_axon_ntff_profile_hook():
        # lazy: only touch the axon .so when tracing is actually requested
        if not mod._resolved:
            mod._resolved = True
            try:
                from trn_agent_boot.trn_boot import _ntff_profile_via_ctypes

                so_path = "/opt/axon/libaxon_pjrt.so"
                if os.path.exists(so_path):
                    mod._hook = _ntff_profile_via_ctypes(so_path)
            except Exception:
                mod._hook = None
        return mod._hook

    mod.set_axon_ntff_profile_hook = set_axon_ntff_profile_hook
    mod.get_axon_ntff_profile_hook = get_axon_ntff_profile_hook
    sys.modules["antenv.axon_hooks"] = mod
    antenv.axon_hooks = mod


try:
    _install_axon_hooks_shim()
except Exception:
    pass





def _build():
    key = (CHUNK, RPP)
    if key in _BUILD_CACHE:
        return _BUILD_CACHE[key]

    nc = bacc.Bacc(
        "TRN2", target_bir_lowering=False, debug=False, num_devices=NCORES
    )
    v1 = nc.dram_tensor("v1", [NF, M], mybir.dt.float32, kind="ExternalInput")
    v2 = nc.dram_tensor("v2", [NF, M], mybir.dt.float32, kind="ExternalInput")
    out = nc.dram_tensor(
        "partial", [P, 2, K], mybir.dt.float32, kind="ExternalOutput"
    )

    # [nf, m] -> [p, t, m]: flat tile t holds flat rows t*128 .. t*128+127,
    # one 4 KiB contiguous line per partition
    v1r = v1.ap().rearrange("(t p) m -> p t m", p=P)
    v2r = v2.ap().rearrange("(t p) m -> p t m", p=P)
    out_r = out.ap()  # [p, c, n]: per-partition 2 KiB contiguous lines

    with tile.TileContext(nc) as tc:
        with (
            tc.tile_pool(name="io", bufs=3) as io_pool,
            tc.tile_pool(name="cv", bufs=3) as cv_pool,
            tc.tile_pool(name="acc", bufs=1, space="PSUM") as psum_pool,
            tc.tile_pool(name="res", bufs=1) as res_pool,
        ):
            # one PSUM region per 128-row half of the [256, 256] output
            ps0 = psum_pool.tile([P, K], mybir.dt.float32)
            ps1 = psum_pool.tile([P, K], mybir.dt.float32)

            # one DMA pair per flat tile on a single queue: strict FIFO
            # keeps the v1/v2 streams in lockstep (two queues skew ~2.5us
            # apart, stalling every tile's matmuls on the late one).
            # Per-slot casts let the matmuls chase each tile's landing, so
            # only the final tile's compute trails the stream.
            for t in range(NT):
                last_t = t == NT - 1
                raw1 = io_pool.tile([P, M], mybir.dt.float32, tag="r1")
                raw2 = io_pool.tile([P, M], mybir.dt.float32, tag="r2")
                nc.sync.dma_start(raw1[:], v1r[:, t, :])
                if not last_t:
                    nc.sync.dma_start(raw2[:], v2r[:, t, :])
                else:
                    # final tile: v2 lands in halves so the last slots'
                    # casts + matmuls trail the stream minimally
                    h = M // 2
                    nc.sync.dma_start(raw2[:, 0:h], v2r[:, t, 0:h])
                    nc.sync.dma_start(raw2[:, h:M], v2r[:, t, h:M])
                for s in range(RPP):
                    sk = slice(s * K, (s + 1) * K)
                    # matmul inputs must be rounded by a compute op: cast
                    # v1 on ACT / v2 on DVE (final tile alternates v2
                    # slots across DVE/ACT to halve the cast chain)
                    t1 = cv_pool.tile([P, K], mybir.dt.bfloat16, tag=f"c1{s}")
                    t2 = cv_pool.tile([P, K], mybir.dt.bfloat16, tag=f"c2{s}")
                    nc.scalar.copy(t1[:], raw1[:, sk])
                    if last_t and s % 2 == 1:
                        nc.scalar.copy(t2[:], raw2[:, sk])
                    else:
                        nc.vector.tensor_copy(t2[:], raw2[:, sk])
                    fst = t == 0 and s == 0
                    stop = last_t and s == RPP - 1
                    nc.tensor.matmul(
                        ps0[:], t1[:, 0:128], t2[:], start=fst, stop=stop
                    )
                    nc.tensor.matmul(
                        ps1[:], t1[:, 128:256], t2[:], start=fst, stop=stop
                    )

            # ps0 -> res on DVE, ps1 -> res on ACT (parallel), then one
            # 256 KiB output DMA with 2 KiB descriptor lines
            res = res_pool.tile([P, 2, K], mybir.dt.float32, tag="o")
            nc.vector.tensor_copy(res[:, 0, :], ps0[:])
            nc.scalar.copy(res[:, 1, :], ps1[:])
            nc.sync.dma_start(out_r[:], res[:])

    nc.compile()
    _BUILD_CACHE[key] = nc
    return nc


def kernel(latent_view_1, latent_view_2):
    global LAST_RESULT
    v1 = np.ascontiguousarray(np.asarray(latent_view_1, dtype=np.float32))
    v2 = np.ascontiguousarray(np.asarray(latent_view_2, dtype=np.float32))
    assert v1.shape == (N_FULL, K) and v2.shape == (N_FULL, K)

    nc = _build()
    in_maps = [
        {
            "v1": v1[c * N_LOC : (c + 1) * N_LOC].reshape(NF, M),
            "v2": v2[c * N_LOC : (c + 1) * N_LOC].reshape(NF, M),
        }
        for c in range(NCORES)
    ]
    LAST_RESULT = bass_utils.run_bass_kernel_spmd(
        nc, in_maps, core_ids=list(range(NCORES))
    )

    # host epilogue in float64 on the tiny [256, 256] joint
    p_i_j = np.zeros((K, K), dtype=np.float64)
    for r in LAST_RESULT.results:
        part = np.asarray(r["partial"], dtype=np.float64).reshape(P, 2, K)
        p_i_j += part.transpose(1, 0, 2).reshape(K, K)
    p_i_j = (p_i_j + p_i_j.T) / 2.0
    p_i_j = p_i_j / p_i_j.sum()
    p_i = p_i_j.sum(axis=1, keepdims=True)
    p_j = p_i_j.sum(axis=0, keepdims=True)
    p_i_j = np.maximum(p_i_j, EPS)
    p_i = np.maximum(p_i, EPS)
    p_j = np.maximum(p_j, EPS)
    loss = -(
        p_i_j
        * (
            np.log(p_i_j)
            - (ALPHA + 1.0) * np.log(p_j)
            - (ALPHA + 1.0) * np.log(p_i)
        )
    ).sum()
    return np.array(loss, dtype=np.float32)


# revision 22
# speedup vs baseline: 1.0457x; 1.0457x over previous
"""CrossViewContrastiveLoss Trainium2 kernel.

loss = f(v1^T @ v2) where v1, v2 are [131072, 256] fp32 and f is a cheap
normalize/log epilogue on the [256, 256] joint matrix.

Strategy (data-parallel over N across 8 cores):
  - core c computes partial_c = v1[rows]^T @ v2[rows] for its 16384-row
    shard as a PE GEMM streaming 32 MiB of HBM (memory-bound).
  - the shard is viewed as [4096, 1024] so every DMA descriptor line is
    4 KiB contiguous; each 128-partition "flat tile" holds 4 sub-rows
    per partition and contributes 4 rank-128 matmul groups.
  - ALL input DMAs ride ONE queue (sync): strict FIFO keeps v1/v2 in
    lockstep and measurably beats two queues (~385 vs ~330 GB/s when
    HBM is uncontended; two queues also skew ~2.5 us apart, stalling
    matmuls on the late tensor).
  - per-slot bf16 casts (ACT for v1, DVE for v2) let matmuls chase each
    tile's landing; the final tile lands in three pieces with casts
    spread over both engines so almost nothing trails the last byte.
  - the [256, 256] fp32 partial leaves via one 256 KiB DMA; the host
    sums the eight partials in float64 and runs the epilogue (65536
    elements -- negligible next to 256 MiB of streaming).
"""

import os

import numpy as np

import concourse.bacc as bacc
import concourse.bass as bass
import concourse.mybir as mybir
import concourse.tile as tile
from concourse import bass_utils

N_FULL = 131072
K = 256
NCORES = 8
N_LOC = N_FULL // NCORES  # 16384 rows per core
P = 128
RPP = int(os.environ.get("CVCL_RPP", "4"))  # rows packed per partition line
M = RPP * K  # 1024 elements per flat row
NF = N_LOC // RPP  # 4096 flat rows per core
NT = NF // P  # 32 flat tiles of 128 flat rows
CHUNK = int(os.environ.get("CVCL_CHUNK", "2"))  # flat tiles per DMA (max)
ALPHA = 9.0
EPS = 2.220446049250313e-16

_BUILD_CACHE = {}
LAST_RESULT = None  # BassKernelResults of the most recent run (for test.py)


def _install_axon_hooks_shim():
    """bass_utils' trace path imports antenv.axon_hooks, which this image
    lacks. Provide it, wiring the ctypes NTFF hook from trn_boot when the
    axon .so supports it. Harmless no-op when tracing is off."""
    import sys
    import types

    try:
        from antenv import axon_hooks  # noqa: F401

        return
    except ImportError:
        pass
    try:
        import antenv
    except ImportError:
        return
    mod = types.ModuleType("antenv.axon_hooks")
    mod._hook = None
    mod._resolved = False

    def set_axon_ntff_profile_hook(h):
        mod._hook = h
        mod._resolved = True

    def get_axon_ntff_profile_hook():
        # lazy: only touch the axon .so when tracing is actually requested
        if not mod._resolved:
            mod._resolved = True
            try:
                from trn_agent_boot.trn_boot import _ntff_profile_via_ctypes

                so_path = "/opt/axon/libaxon_pjrt.so"
                if os.path.exists(so_path):
                    mod._hook = _ntff_profile_via_ctypes(so_path)
            except Exception:
                mod._hook = None
        return mod._hook

    mod.set_axon_ntff_profile_hook = set_axon_ntff_profile_hook
    mod.get_axon_ntff_profile_hook = get_axon_ntff_profile_hook
    sys.modules["antenv.axon_hooks"] = mod
    antenv.axon_hooks = mod


try:
    _install_axon_hooks_shim()
except Exception:
    pass





def _build():
    key = (CHUNK, RPP)
    if key in _BUILD_CACHE:
        return _BUILD_CACHE[key]

    nc = bacc.Bacc(
        "TRN2", target_bir_lowering=False, debug=False, num_devices=NCORES
    )
    v1 = nc.dram_tensor("v1", [NF, M], mybir.dt.float32, kind="ExternalInput")
    v2 = nc.dram_tensor("v2", [NF, M], mybir.dt.float32, kind="ExternalInput")
    out = nc.dram_tensor(
        "partial", [P, 2, K], mybir.dt.float32, kind="ExternalOutput"
    )

    # [nf, m] -> [p, t, m]: flat tile t holds flat rows t*128 .. t*128+127,
    # one 4 KiB contiguous line per partition
    v1r = v1.ap().rearrange("(t p) m -> p t m", p=P)
    v2r = v2.ap().rearrange("(t p) m -> p t m", p=P)
    out_r = out.ap()  # [p, c, n]: per-partition 2 KiB contiguous lines

    with tile.TileContext(nc) as tc:
        with (
            tc.tile_pool(name="io", bufs=3) as io_pool,
            tc.tile_pool(name="cv", bufs=3) as cv_pool,
            tc.tile_pool(name="acc", bufs=1, space="PSUM") as psum_pool,
            tc.tile_pool(name="res", bufs=1) as res_pool,
        ):
            # one PSUM region per 128-row half of the [256, 256] output
            ps0 = psum_pool.tile([P, K], mybir.dt.float32)
            ps1 = psum_pool.tile([P, K], mybir.dt.float32)

            # one DMA pair per flat tile on a single queue: strict FIFO
            # keeps the v1/v2 streams in lockstep (two queues skew ~2.5us
            # apart, stalling every tile's matmuls on the late one).
            # Per-slot casts let the matmuls chase each tile's landing, so
            # only the final tile's compute trails the stream.
            for t in range(NT):
                last_t = t == NT - 1
                raw1 = io_pool.tile([P, M], mybir.dt.float32, tag="r1")
                raw2 = io_pool.tile([P, M], mybir.dt.float32, tag="r2")
                nc.sync.dma_start(raw1[:], v1r[:, t, :])
                if not last_t:
                    nc.sync.dma_start(raw2[:], v2r[:, t, :])
                else:
                    # final tile: v2 lands in three pieces (last two are
                    # single slots) so after the last 128 KiB only one
                    # cast + matmul pair remain
                    h = M - 2 * K
                    q = M - K
                    nc.sync.dma_start(raw2[:, 0:h], v2r[:, t, 0:h])
                    nc.sync.dma_start(raw2[:, h:q], v2r[:, t, h:q])
                    nc.sync.dma_start(raw2[:, q:M], v2r[:, t, q:M])
                for s in range(RPP):
                    sk = slice(s * K, (s + 1) * K)
                    # matmul inputs must be rounded by a compute op: cast
                    # v1 on ACT / v2 on DVE. The final tile spreads both
                    # tensors' casts across DVE+ACT so neither engine's
                    # cast backlog trails the stream end.
                    t1 = cv_pool.tile([P, K], mybir.dt.bfloat16, tag=f"c1{s}")
                    t2 = cv_pool.tile([P, K], mybir.dt.bfloat16, tag=f"c2{s}")
                    if not last_t:
                        nc.scalar.copy(t1[:], raw1[:, sk])
                        nc.vector.tensor_copy(t2[:], raw2[:, sk])
                    else:
                        if s % 2 == 0:
                            nc.vector.tensor_copy(t1[:], raw1[:, sk])
                        else:
                            nc.scalar.copy(t1[:], raw1[:, sk])
                        if s >= RPP - 2 or s % 2 == 1:
                            nc.vector.tensor_copy(t2[:], raw2[:, sk])
                        else:
                            nc.scalar.copy(t2[:], raw2[:, sk])
                    fst = t == 0 and s == 0
                    stop = last_t and s == RPP - 1
                    nc.tensor.matmul(
                        ps0[:], t1[:, 0:128], t2[:], start=fst, stop=stop
                    )
                    nc.tensor.matmul(
                        ps1[:], t1[:, 128:256], t2[:], start=fst, stop=stop
                    )

            # ps0 -> res on DVE, ps1 -> res on ACT (parallel), then one
            # 256 KiB output DMA with 2 KiB descriptor lines
            res = res_pool.tile([P, 2, K], mybir.dt.float32, tag="o")
            nc.vector.tensor_copy(res[:, 0, :], ps0[:])
            nc.scalar.copy(res[:, 1, :], ps1[:])
            nc.sync.dma_start(out_r[:], res[:])

    nc.compile()
    _BUILD_CACHE[key] = nc
    return nc


def kernel(latent_view_1, latent_view_2):
    global LAST_RESULT
    v1 = np.ascontiguousarray(np.asarray(latent_view_1, dtype=np.float32))
    v2 = np.ascontiguousarray(np.asarray(latent_view_2, dtype=np.float32))
    assert v1.shape == (N_FULL, K) and v2.shape == (N_FULL, K)

    nc = _build()
    in_maps = [
        {
            "v1": v1[c * N_LOC : (c + 1) * N_LOC].reshape(NF, M),
            "v2": v2[c * N_LOC : (c + 1) * N_LOC].reshape(NF, M),
        }
        for c in range(NCORES)
    ]
    LAST_RESULT = bass_utils.run_bass_kernel_spmd(
        nc, in_maps, core_ids=list(range(NCORES))
    )

    # host epilogue in float64 on the tiny [256, 256] joint
    p_i_j = np.zeros((K, K), dtype=np.float64)
    for r in LAST_RESULT.results:
        part = np.asarray(r["partial"], dtype=np.float64).reshape(P, 2, K)
        p_i_j += part.transpose(1, 0, 2).reshape(K, K)
    p_i_j = (p_i_j + p_i_j.T) / 2.0
    p_i_j = p_i_j / p_i_j.sum()
    p_i = p_i_j.sum(axis=1, keepdims=True)
    p_j = p_i_j.sum(axis=0, keepdims=True)
    p_i_j = np.maximum(p_i_j, EPS)
    p_i = np.maximum(p_i, EPS)
    p_j = np.maximum(p_j, EPS)
    loss = -(
        p_i_j
        * (
            np.log(p_i_j)
            - (ALPHA + 1.0) * np.log(p_j)
            - (ALPHA + 1.0) * np.log(p_i)
        )
    ).sum()
    return np.array(loss, dtype=np.float32)
